# revision 34
# baseline (speedup 1.0000x reference)
"""CharEmbeddingCNN Trainium2 kernel (fp8 alphabet-table formulation).

Reference computation (per word of L=20 chars):
    xe = emb[x]                       # [L, 256] -> treated as [256, L]
    y_k = conv1d_valid(xe, w_k) + b_k # k in (3,4,5), 256 -> 256 channels
    out = relu(max over all (k, t) of y_k[:, t]) * (len != 0)

Key algebraic identity: y_k[o, t] = sum_dk A_{k,dk}[x[t+dk], o] where
A_{k,dk} = emb @ w_k[:, :, dk].T is a [256 alphabet, 256 out] table
precomputed on host in float64. The conv becomes a matmul of the A
tables against one-hot character encodings - and one-hot values are
EXACT in fp8, so only the tables carry quantization error (~1e-2 rel
after max-pooling concentration, vs the 2e-2 bar).

Strategy (data-parallel over 8 NeuronCores, 1024 words each):
  - Host builds per-core one-hot tensors [128 part = c%128, c//128, word,
    t] in fp8e4m3 and the 12 (k,dk) tables scaled x4096 (fp8 subnormal
    avoidance); inverse scale folds into the final relu/mask scale.
  - Conv = PSUM-accumulated DoubleRow fp8 matmuls: the two 128-row
    alphabet halves fuse into one 256-deep matmul at 1 col/cycle - 2x
    the bf16 MAC rate (157 TF/s, measured; the cost model's 0.5 cyc/col
    does not materialize on HW, contiguous or strided). Groups of 28
    words (28*18 = 504 <= 512 PSUM bank limit), 24 matmuls per group,
    no gathers anywhere. This is the PE floor: column-streams =
    MACs / (256 K * 128 out) = 413.7K cycles/core.
  - Biases ride inside the matmuls: alphabet row 0 is the PAD row (all
    table entries zero), so every one-hot column carries a constant 1
    there and row 0 of the dk=0 / dk=1 tables holds fp8(b*S) and its
    fp8 residual - PSUM accumulates y + b with no bias ops anywhere.
  - Max over (k, t) split across engines: ACT copies k=4/k=5 PSUM to
    SBUF bf16; DVE does k=3 reduce_max direct from PSUM plus one
    2x-rate bf16 reduce_max over the copied pair and a tensor_max into
    the bf16 C accumulator. The last group uses an all-DVE path so no
    ACT hop sits after the final matmul.
  - No on-device transpose: finished 128-word C blocks get relu*mask
    fused in one in-place DVE scalar_tensor_tensor (max 0, mult
    mask/4096) and stream out channel-major bf16; the host does the
    final [chan, word] -> [word, chan] transpose + f32 cast. This keeps
    the PE stream 99.6% dense (no transpose head-of-line blocking) and
    halves output DMA.
"""

import numpy as np
import ml_dtypes
from contextlib import ExitStack

import concourse.bacc as bacc
import concourse.tile as tile
from concourse import mybir
from concourse.bass_utils import run_bass_kernel_spmd

F32 = mybir.dt.float32
BF16 = mybir.dt.bfloat16
FP8 = mybir.dt.float8e4
NP_FP8 = ml_dtypes.float8_e4m3

B, S, L = 64, 128, 20
EMB = 256
KS = (3, 4, 5)
NKDK = sum(KS)                 # 12 packed (k, dk) table slices
NCORES = 8
W = (B * S) // NCORES          # words per core (1024)
GW = 28                        # words per matmul group (28*18 <= 512 psum)
SCALE = 4096.0                 # fp8 table scale (keeps values normal)
WARMUP_MM = 36


def _kdk_off(ki, dk):
    return sum(KS[:ki]) + dk


def build_bass(words=W):
    assert words % 128 == 0
    nwb = words // 128
    # Full-width groups + one short remainder. (Splitting the remainder into
    # two smaller groups shortens the post-final-matmul reduction tail by
    # ~1us but costs ~1.5us of extra per-group PE overhead - net negative.)
    groups = [(g * GW, min(GW, words - g * GW))
              for g in range((words + GW - 1) // GW)]

    nc = bacc.Bacc(
        "TRN2",
        target_bir_lowering=False,
        debug=False,
        enable_asserts=False,
    )

    oh_d = nc.dram_tensor("oh", [128, 2 * words * L], FP8,
                          kind="ExternalInput").ap()
    wta_d = nc.dram_tensor("wta", [256, NKDK * 256], FP8,
                           kind="ExternalInput").ap()
    # relu mask (len != 0) / SCALE, broadcast across partitions
    maskf_d = nc.dram_tensor("maskf", [128, words], BF16,
                             kind="ExternalInput").ap()
    # channel-major bf16 output; the host transposes to [word, chan] f32
    out_d = nc.dram_tensor("out", [EMB, words], BF16,
                           kind="ExternalOutput").ap()

    with tile.TileContext(nc) as tc, ExitStack() as ctx:
        const_pool = ctx.enter_context(tc.tile_pool(name="const", bufs=1))
        psum_pool = ctx.enter_context(tc.tile_pool(name="ps", bufs=2,
                                                   space="PSUM"))
        t45_pool = ctx.enter_context(tc.tile_pool(name="t45", bufs=3))
        m3_pool = ctx.enter_context(tc.tile_pool(name="m3", bufs=3))

        # DMA issue split across the two HWDGE sequencers so the two
        # PE-critical transfers (group 0's one-hot slice on Sync, the tables
        # on ACT) issue in parallel; everything else follows in word order.
        oh_t = const_pool.tile([128, 2, words, L], FP8)
        ohv_d = oh_d.rearrange("p (c w t) -> p c w t", c=2, t=L)
        nc.sync.dma_start(oh_t[:, :, 0:GW, :], ohv_d[:, :, 0:GW, :])

        wt = const_pool.tile([128, 2, NKDK, 256], FP8)
        wv = wta_d.rearrange("(c p) ko -> p c ko", c=2)
        nc.scalar.dma_start(wt[:].rearrange("p c k o -> p c (k o)"), wv)

        maskf = const_pool.tile([128, words], BF16)

        # PE p-state warm-up while input DMAs drain
        scratch = const_pool.tile([128, 256], FP8)
        nc.vector.memset(scratch[:], 0.0)
        warm = psum_pool.tile([128, 504], F32, tag="ps0")
        for _ in range(WARMUP_MM):
            nc.tensor.matmul(warm[:, 0:128], scratch[:, 0:128],
                             scratch[:, 0:128], start=True, stop=True)

        C = [const_pool.tile([128, words], BF16, tag=f"c{oc}", name=f"c{oc}")
             for oc in range(2)]

        # Emission boundaries: 128-word blocks, except the final block is
        # split so only the last (short) group's words sit on the tail chain
        # after the final matmul.
        bounds = [128 * i for i in range(1, nwb)]
        last_w0 = groups[-1][0]
        if last_w0 > bounds[-1]:
            bounds.append(last_w0)
        bounds.append(words)
        bi = [0, 0]
        covered = [0, 0]

        def emit_ready(oc):
            while bi[oc] < len(bounds) and covered[oc] >= bounds[bi[oc]]:
                lo = bounds[bi[oc] - 1] if bi[oc] else 0
                sl = slice(lo, bounds[bi[oc]])
                # relu * mask fused, in place on the finished C piece
                nc.vector.scalar_tensor_tensor(
                    C[oc][:, sl], C[oc][:, sl], 0.0, maskf[:, sl],
                    op0=mybir.AluOpType.max,
                    op1=mybir.AluOpType.mult)
                nc.sync.dma_start(
                    out_d[oc * 128:(oc + 1) * 128, sl], C[oc][:, sl])
                bi[oc] += 1

        for gi, (w0, nw) in enumerate(groups):
            if gi == 1:
                # non-critical input DMAs deferred past group 0 so the first
                # conv matmul's semaphore wait covers only its own two DMAs
                nc.sync.dma_start(oh_t[:, :, GW:128, :],
                                  ohv_d[:, :, GW:128, :])
                nc.scalar.dma_start(maskf[:], maskf_d[:])
                for wc in range(1, nwb):
                    nc.sync.dma_start(
                        oh_t[:, :, wc * 128:(wc + 1) * 128, :],
                        ohv_d[:, :, wc * 128:(wc + 1) * 128, :])
            for oc in range(2):
                ps = []
                for ki, k in enumerate(KS):
                    lk = L - k + 1
                    p = psum_pool.tile([128, nw, lk], F32, tag=f"ps{ki}",
                                       name=f"ps{ki}")
                    for dk in range(k):
                        nc.tensor.matmul(
                            p[:],
                            wt[:, :, _kdk_off(ki, dk),
                               oc * 128:(oc + 1) * 128],
                            oh_t[:, :, w0:w0 + nw, dk:dk + lk],
                            start=(dk == 0), stop=(dk == k - 1),
                            perf_mode=mybir.MatmulPerfMode.DoubleRow,
                        )
                    ps.append(p)
                # ACT evacuates k=4/k=5 with fused bias-add (bf16, packed
                # so the DVE reduce below runs in 2x_1p mode)
                cs = C[oc][:, w0:w0 + nw]
                m3 = m3_pool.tile([128, nw], F32, tag="m3", name="m3")
                nc.vector.reduce_max(m3[:], ps[0][:],
                                     axis=mybir.AxisListType.X)
                if (w0, nw) == groups[-1]:
                    # short all-DVE tail: no ACT hop after the last matmul
                    m34 = m3_pool.tile([128, nw], F32, tag="m34", name="m34")
                    nc.vector.reduce_max(m34[:], ps[1][:],
                                         axis=mybir.AxisListType.X)
                    nc.vector.tensor_max(m34[:], m34[:], m3[:])
                    nc.vector.reduce_max(cs, ps[2][:],
                                         axis=mybir.AxisListType.X)
                    nc.vector.tensor_max(cs, cs, m34[:])
                else:
                    t45 = t45_pool.tile([128, nw, 33], BF16, tag="t45",
                                        name="t45")
                    nc.scalar.activation(
                        t45[:, :, 0:17], ps[1][:],
                        mybir.ActivationFunctionType.Copy)
                    nc.scalar.activation(
                        t45[:, :, 17:33], ps[2][:],
                        mybir.ActivationFunctionType.Copy)
                    nc.vector.reduce_max(cs, t45[:],
                                         axis=mybir.AxisListType.X)
                    nc.vector.tensor_max(cs, cs, m3[:])
                covered[oc] = w0 + nw
                emit_ready(oc)
        assert covered == [words, words] and bi == [len(bounds)] * 2

    nc.compile()
    return nc


def prep_shared(emb, w3, w4, w5, b3, b4, b5):
    """Tables with biases baked into alphabet row 0 (the PAD row, whose
    true entries are all zero): every one-hot column carries a constant 1 in
    row 0, so the dk=0 table row 0 adds fp8(b*S) and the dk=1 row adds the
    fp8 residual - bias lands exactly once per position, at ~f32 accuracy."""
    emb64 = np.asarray(emb).astype(np.float64)
    wta = np.empty((256, NKDK, 256), dtype=NP_FP8)
    for ki, (w, b) in enumerate(((w3, b3), (w4, b4), (w5, b5))):
        bs = np.asarray(b).astype(np.float64) * SCALE
        q0 = bs.astype(NP_FP8).astype(np.float64)
        row0 = {0: q0, 1: bs - q0}
        for dk in range(KS[ki]):
            # wta[c, off, o] = SCALE * sum_i emb[c, i] w[o, i, dk]
            A = emb64 @ np.asarray(w)[:, :, dk].astype(np.float64).T * SCALE
            A[0, :] = row0.get(dk, 0.0)
            wta[:, _kdk_off(ki, dk), :] = A.astype(NP_FP8)
    return np.ascontiguousarray(wta.reshape(256, NKDK * 256))


def prep_core(xf, lensf, words=W):
    """Per-core one-hot + mask packing. xf: [words, L], lensf: [words]."""
    oh = np.zeros((256, words * L), dtype=NP_FP8)
    oh[xf.reshape(-1), np.arange(words * L)] = 1.0
    oh[0, :] = 1.0  # bias marker row (also covers the zero PAD rows)
    oh = oh.reshape(2, 128, words * L).transpose(1, 0, 2)  # [128, c_hi, w*t]
    maskf = np.broadcast_to(
        ((lensf != 0).astype(np.float32) / SCALE).astype(ml_dtypes.bfloat16),
        (128, words))
    return (np.ascontiguousarray(oh).reshape(128, 2 * words * L),
            np.ascontiguousarray(maskf))


_CACHE = {}


def _get_nc(words=W):
    if words not in _CACHE:
        _CACHE[words] = build_bass(words)
    return _CACHE[words]


def run(x, lens, emb, w3, b3, w4, b4, w5, b5, trace=False, **spmd_kwargs):
    x = np.asarray(x)
    lens = np.asarray(lens)
    nc = _get_nc()
    wta = prep_shared(emb, w3, w4, w5, b3, b4, b5)
    xf = x.reshape(B * S, L)
    lensf = lens.reshape(B * S)
    in_maps = []
    for c in range(NCORES):
        sl = slice(c * W, (c + 1) * W)
        oh, maskf = prep_core(xf[sl], lensf[sl])
        in_maps.append({
            "oh": oh, "wta": wta, "maskf": maskf,
        })
    res = run_bass_kernel_spmd(
        nc, in_maps, core_ids=list(range(NCORES)), trace=trace, **spmd_kwargs)
    # per-core [256 chan, words] bf16 -> [B, S, 256] f32
    out = np.concatenate([r["out"] for r in res.results], axis=1)
    out = out.T.astype(np.float32)
    return np.ascontiguousarray(out.reshape(B, S, EMB)), res


def kernel(x, lens, emb, w3, b3, w4, b4, w5, b5, **unused):
    out, _ = run(x, lens, emb, w3, b3, w4, b4, w5, b5)
    return out
